# revision 72
# baseline (speedup 1.0000x reference)
"""GroupedQueryAttention Trainium2 Bass kernel (v2).

Sharding: 8 cores = (B=2) x (G=4 KV groups). Each core computes, for its
(batch b, kv-group g): the 4 query heads' Q/K/V projections, causal flash
attention, and a partial output projection Y^T_g (bf16). Host sums the 4
partials per batch and adds an adjusted bias (bo + bv-term folded in).

Key structure (all transposed: token dim T on the free axis):
  xT[d, t]     uploaded pre-transposed from host (bf16) - no PE transposes
  Q^T, K^T     from projection matmuls (W chunk stationary, xT moving)
  V^T -> V     PE transpose per 128-block, staged in the st PSUM slots
  S^T[s, t]  = (K^T s-block).T @ Q^T        (one 128-wide matmul per s-block)
  P^T        = exp(scale * S^T + mask)      (ACT, PSUM -> SBUF, bf16)
  O^T[dh, t] += (V s-block).T @ P^T         (PSUM accumulation over s-blocks)
  rowsum     += ones.T @ P^T                (PSUM accumulation, M=1)
  Y^T[dm, t] = sum_c (Wo chunk).T @ O^T_c   (per 128-row dm block, bf16 out)

The V bias never enters the kernel: O = (P@(V0+1*bv))/rowsum = P@V0/rowsum
+ bv, and the constant bv contribution to Y is folded into bo on the host.

oproj(tau-1) m-blocks are interleaved into flash(tau) as PE filler so the
S->exp->PV dependency chain's bubbles are absorbed by dense matmul work.

Normalize chain: rs copy (scalar) -> reciprocal_approx_fast [1,512] (DVE)
-> partition_broadcast (gpsimd) -> mul (DVE). Avoids the 3.3us exact
reciprocal.
"""

import sys

sys.path.insert(0, "/opt/trn_rl_repo")

from contextlib import ExitStack

import ml_dtypes
import numpy as np

import concourse.bass as bass  # noqa: F401
import concourse.tile as tile
from concourse import bacc, bass_isa, mybir
from concourse.bass_utils import run_bass_kernel_spmd

F32 = mybir.dt.float32
BF16 = mybir.dt.bfloat16
AF = mybir.ActivationFunctionType

D = 2048          # model dim
T = 2048          # tokens
DH = 128          # head dim
G = 4             # kv groups
HPG = 4           # query heads per group
QC = HPG * DH     # query cols per group = 512
ND = D // 128     # 16 contraction chunks
NTAU = 4          # t tiles of 512
TW = 512          # t tile width
SCALE = DH ** -0.5
NEG = -1e30

TRACE = False
TRACE_KW = {}
LAST_RESULTS = None

_CACHE = {}


def _body(ctx, tc, xT, wq, wk, wv, wo, bq, bk, maskTd, identd, yT):
    nc = tc.nc

    # PSUM (16KB/partition exactly): acc 2x2KB + st-pair 2x4KB (shared with
    # V-transpose staging) + ot 2x2KB
    psacc = ctx.enter_context(tc.tile_pool(name="psacc", bufs=2, space="PSUM"))
    psst = ctx.enter_context(tc.tile_pool(name="psst", bufs=2, space="PSUM"))
    psot = ctx.enter_context(tc.tile_pool(name="psot", bufs=2, space="PSUM"))

    consts = ctx.enter_context(tc.tile_pool(name="consts", bufs=1))
    qkv = ctx.enter_context(tc.tile_pool(name="qkv", bufs=1))
    xtp = ctx.enter_context(tc.tile_pool(name="xtp", bufs=ND))
    wkp = ctx.enter_context(tc.tile_pool(name="wkp", bufs=ND))
    wvp = ctx.enter_context(tc.tile_pool(name="wvp", bufs=ND))
    wqp = ctx.enter_context(tc.tile_pool(name="wqp", bufs=ND))
    wop = ctx.enter_context(tc.tile_pool(name="wop", bufs=1))
    vts = ctx.enter_context(tc.tile_pool(name="vstage", bufs=2))
    ptp = ctx.enter_context(tc.tile_pool(name="ptp", bufs=3))
    ptsums = ctx.enter_context(tc.tile_pool(name="ptsums", bufs=2))
    nrm = ctx.enter_context(tc.tile_pool(name="norm", bufs=2))
    otp_pool = ctx.enter_context(tc.tile_pool(name="otsb", bufs=1))
    yb = ctx.enter_context(tc.tile_pool(name="ybounce", bufs=3))

    # ---- constants on the scalar queue (small, early); wo also scalar
    # (needed late; the Activation HWDGE queue gets starved in arbitration
    # so nothing startup-critical goes on it)
    tri01 = consts.tile([128, 128], BF16, tag="tri01")
    nc.scalar.dma_start(tri01, maskTd)
    bqt = consts.tile([128, 4], F32, tag="bqt")
    nc.scalar.dma_start(bqt, bq.rearrange("(c p) -> p c", p=128))
    bkt = consts.tile([128, 1], F32, tag="bkt")
    nc.scalar.dma_start(bkt, bk.rearrange("(c p) -> p c", p=128))
    ident = consts.tile([128, 128], BF16, tag="ident")
    nc.scalar.dma_start(ident, identd)

    # ---- weights + x on the two fast queues (sync HWDGE, gpsimd SWDGE),
    # strictly in first-use order: wk, x(sg0), wv, wq, x(sg1..3).
    xts = [xtp.tile([128, T], BF16, tag="xt", name=f"xt{d}") for d in range(ND)]
    wkts = [wkp.tile([128, DH], BF16, tag="wk", name=f"wk{d}") for d in range(ND)]
    wvts = [wvp.tile([128, DH], BF16, tag="wv", name=f"wv{d}") for d in range(ND)]
    wqts = [wqp.tile([128, QC], BF16, tag="wq", name=f"wq{d}") for d in range(ND)]
    wot = [wop.tile([128, D], BF16, tag=f"wo{c}", name=f"wo{c}") for c in range(HPG)]

    qlist = [nc.sync, nc.gpsimd]
    qi = 0

    def q_next():
        nonlocal qi
        eng = qlist[qi % 2]
        qi += 1
        return eng

    for d in range(ND):
        q_next().dma_start(wkts[d], wk[d * 128:(d + 1) * 128, :])
        q_next().dma_start(xts[d][:, 0:TW], xT[d * 128:(d + 1) * 128, 0:TW])
    for d in range(ND):
        q_next().dma_start(wvts[d], wv[d * 128:(d + 1) * 128, :])
    for d in range(ND):
        q_next().dma_start(wqts[d], wq[d * 128:(d + 1) * 128, :])
    for sg in range(1, NTAU):
        for d in range(ND):
            q_next().dma_start(
                xts[d][:, sg * TW:(sg + 1) * TW],
                xT[d * 128:(d + 1) * 128, sg * TW:(sg + 1) * TW])
    for c in range(HPG):
        nc.scalar.dma_start(wot[c], wo[c * 128:(c + 1) * 128, :])

    # ---- HAM warm-up: real matmuls on a memset tile (no DMA dependency)
    # while the x DMAs land, so the PE clock-gate is already at 8/8 when the
    # K projection starts (PE-mode transposes don't count as HAM activity)
    warm_in = consts.tile([128, 128], BF16, tag="warm_in")
    nc.vector.memset(warm_in, 0.0)

    def warm_fill(n):
        for w in range(n):
            wps = psot.tile([128, 128], F32, tag="ot", name="warm")
            nc.tensor.matmul(wps, warm_in, warm_in, start=True, stop=True)

    warm_fill(72)

    qt = [qkv.tile([128, T], BF16, tag=f"qt{j}", name=f"qt{j}") for j in range(HPG)]
    kt = qkv.tile([128, T], BF16, tag="kt")
    vv = qkv.tile([128, ND, 128], BF16, tag="vv")  # [s%128, s_block, dh]

    # ---- K + V projection for one sg column block
    def kvproj(sg):
        ps = psacc.tile([128, TW], F32, tag="acc", name="psk")
        for d in range(ND):
            nc.tensor.matmul(ps, wkts[d], xts[d][:, sg * TW:(sg + 1) * TW],
                             start=(d == 0), stop=(d == ND - 1))
        nc.scalar.activation(kt[:, sg * TW:(sg + 1) * TW], ps, AF.Identity,
                             bias=bkt[:, 0:1])

        ps2 = psacc.tile([128, TW], F32, tag="acc", name="psv")
        for d in range(ND):
            nc.tensor.matmul(ps2, wvts[d], xts[d][:, sg * TW:(sg + 1) * TW],
                             start=(d == 0), stop=(d == ND - 1))
        vtt = vts.tile([128, TW], BF16, tag="vt")
        nc.scalar.copy(vtt, ps2)
        # V^T -> V native, staged in an st-tag PSUM slot (idle until flash)
        stg = psst.tile([128, TW], BF16, tag="st", name="vstg")
        for i in range(4):
            nc.tensor.transpose(stg[:, i * 128:(i + 1) * 128],
                                vtt[:, i * 128:(i + 1) * 128], ident)
        nc.vector.tensor_copy(vv[:, sg * 4:(sg + 1) * 4, :], stg)

    # ---- Q projection for one t-tile (4 head blocks)
    def qproj(tau):
        for cb in range(HPG):
            ps = psacc.tile([128, TW], F32, tag="acc", name="psq")
            for d in range(ND):
                nc.tensor.matmul(
                    ps, wqts[d][:, cb * 128:(cb + 1) * 128],
                    xts[d][:, tau * TW:(tau + 1) * TW],
                    start=(d == 0), stop=(d == ND - 1))
            nc.scalar.activation(qt[cb][:, tau * TW:(tau + 1) * TW], ps,
                                 AF.Identity, bias=bqt[:, cb:cb + 1])

    # ---- output projection m-block (4 matmuls + copy + store)
    ots = [otp_pool.tile([128, T], BF16, tag=f"ot{j}", name=f"ots{j}")
           for j in range(HPG)]

    def oproj_block(tau, m):
        yp = psacc.tile([128, TW], F32, tag="acc", name="yp")
        for c in range(HPG):
            nc.tensor.matmul(
                yp, wot[c][:, m * 128:(m + 1) * 128],
                ots[c][:, tau * TW:(tau + 1) * TW],
                start=(c == 0), stop=(c == HPG - 1))
        ys = yb.tile([128, TW], BF16, tag="y", name="ys")
        nc.scalar.copy(ys, yp)
        nc.sync.dma_start(
            yT[m * 128:(m + 1) * 128, tau * TW:(tau + 1) * TW], ys)

    # ---- phase C: per-tau pipeline [K,V,Q, flash(+oproj filler)].
    # The PV/P-sum consume pipeline runs 2 pairs behind the S/exp front and
    # is flattened ACROSS head and tau boundaries (pend FIFO), so tail exp
    # latency always hides under the next head's (or next tau's K/V/Q) PE
    # work. normalize(j) is emitted right after that head's last consume.

    # entry: [head dict h, q]
    pend = []
    # deferred normalize tails [h, rc]: the gpsimd all-reduce is issued at
    # the head's last consume, but the DVE recip+mul are emitted two consume
    # slots later so the DVE FIFO never blocks waiting on the all-reduce
    norm_pend = []

    cc = [0]
    norms_done = [0] * NTAU

    def norm_flush(drain=False):
        while norm_pend and (drain or norm_pend[0][2] <= cc[0]):
            h, rc, _ = norm_pend.pop(0)
            rcr = nrm.tile([128, TW], F32, tag="rcr")
            nc.vector.reciprocal_approx_fast(rcr, rc)
            nc.vector.tensor_mul(
                ots[h["j"]][:, h["tau"] * TW:(h["tau"] + 1) * TW],
                h["otp"], rcr)
            norms_done[h["tau"]] += 1

    def consume_one():
        cc[0] += 1
        norm_flush()
        h, q = pend.pop(0)
        otp, ptsum, pairs, npair_ = (
            h["otp"], h["ptsum"], h["pairs"], h["npair"])
        _, ptq, lo0, lo1 = pairs[q]
        first = h["ncons"] == 0
        h["ncons"] += 1
        last = h["ncons"] == npair_
        nc.tensor.matmul(otp[:, lo0:], vv[:, 2 * q, :],
                         ptq[:, 0, lo0:], start=first, stop=False)
        nc.tensor.matmul(otp[:, lo1:], vv[:, 2 * q + 1, :],
                         ptq[:, 1, lo1:], start=False, stop=last)
        # denominator: per-partition P-sums accumulate on DVE (bf16)
        if first:
            nc.vector.tensor_copy(ptsum, ptq[:, 0, :])
        else:
            nc.vector.tensor_add(ptsum[:, lo0:], ptsum[:, lo0:],
                                 ptq[:, 0, lo0:])
        nc.vector.tensor_add(ptsum[:, lo1:], ptsum[:, lo1:], ptq[:, 1, lo1:])
        del pairs[q]
        if last:
            # 128-way cross-partition sum in f32 on gpsimd (its own queue);
            # the DVE recip+mul are deferred two consume slots
            rc = nrm.tile([128, TW], F32, tag="rc")
            nc.gpsimd.partition_all_reduce(rc, ptsum, 128,
                                           bass_isa.ReduceOp.add)
            norm_pend.append([h, rc, cc[0] + 3])

    filler = []
    for tau in range(NTAU):
        kvproj(tau)
        qproj(tau)
        nsb = 4 * tau + 4
        npair = nsb // 2
        # filler units: oproj(tau-1) m-blocks spread over this tau's steps;
        # leftovers carry over instead of flushing densely at the tau
        # boundary (where they collide with the next K chain on the acc pool)
        if tau > 0:
            filler = filler + [(tau - 1, m) for m in range(ND)]
        total_steps = HPG * nsb
        fill_every = max(1, total_steps // max(len(filler), 1))
        step = 0
        fi = 0

        for j in range(HPG):
            h = {
                "otp": psot.tile([128, TW], F32, tag="ot", name="otp"),
                "ptsum": ptsums.tile([128, TW], BF16, tag="ps", name="ptsum"),
                "pairs": {}, "tau": tau, "j": j, "npair": npair, "ncons": 0,
            }
            qslice = qt[j][:, tau * TW:(tau + 1) * TW]

            for sb in range(nsb):
                di = sb - 4 * tau
                lo = di * 128 if di >= 0 else 0   # valid t-range start
                if sb % 2 == 0:
                    stp = psst.tile([128, 2, TW], F32, tag="st")
                    ptq = ptp.tile([128, 2, TW], BF16, tag="pt")
                    h["pairs"][sb // 2] = [stp, ptq, lo, lo]
                pr = h["pairs"][sb // 2]
                pr[2 + sb % 2] = lo
                stp = pr[0]
                # plane 1 streams from the pair's lo0 so the batched pair
                # exp never touches bytes this tile didn't write; the
                # causally-invalid strip is masked out of pt after the exp
                slo = pr[2] if sb % 2 == 1 else lo
                nc.tensor.matmul(stp[:, sb % 2, slo:],
                                 kt[:, sb * 128:(sb + 1) * 128],
                                 qslice[:, slo:], start=True, stop=True)
                if sb % 2 == 1:
                    nc.scalar.activation(pr[1][:, :, pr[2]:],
                                         stp[:, :, pr[2]:], AF.Exp,
                                         scale=SCALE)
                    if di >= 0:
                        # causal triangle: zero pt's above-diagonal entries
                        # post-exp (keeps the DVE off the S->exp path)
                        nc.vector.tensor_mul(
                            pr[1][:, 0, pr[2]:pr[2] + 128],
                            pr[1][:, 0, pr[2]:pr[2] + 128], tri01)
                        nc.vector.tensor_mul(
                            pr[1][:, 1, pr[3]:pr[3] + 128],
                            pr[1][:, 1, pr[3]:pr[3] + 128], tri01)
                    pend.append([h, sb // 2])
                    if len(pend) > 2:
                        consume_one()
                step += 1
                # a filler runs only once all 4 of its tau's normalize muls
                # have been emitted, else the oproj MM waits on a mul whose
                # producer sits behind it in PE program order (deadlock)
                # keep the last few steps filler-free so the next tau's K
                # chain doesn't wait on a filler's acc slot at the boundary
                if (fi < len(filler) and step % fill_every == 0
                        and step <= total_steps - 4
                        and norms_done[filler[fi][0]] == HPG):
                    oproj_block(*filler[fi])
                    fi += 1
        filler = filler[fi:]
    for t, m in filler:
        oproj_block(t, m)
    while pend:
        consume_one()
    norm_flush(drain=True)
    for m in range(ND):
        oproj_block(NTAU - 1, m)


def _build_nc():
    if "nc" in _CACHE:
        return _CACHE["nc"]
    nc = bacc.Bacc("TRN2", target_bir_lowering=False, debug=False)
    xT = nc.dram_tensor("xT", [D, T], BF16, kind="ExternalInput").ap()
    wq = nc.dram_tensor("wq", [D, QC], BF16, kind="ExternalInput").ap()
    wk = nc.dram_tensor("wk", [D, DH], BF16, kind="ExternalInput").ap()
    wv = nc.dram_tensor("wv", [D, DH], BF16, kind="ExternalInput").ap()
    wo = nc.dram_tensor("wo", [QC, D], BF16, kind="ExternalInput").ap()
    bq = nc.dram_tensor("bq", [QC], F32, kind="ExternalInput").ap()
    bk = nc.dram_tensor("bk", [DH], F32, kind="ExternalInput").ap()
    maskTd = nc.dram_tensor("maskT", [128, 128], BF16, kind="ExternalInput").ap()
    identd = nc.dram_tensor("ident", [128, 128], BF16, kind="ExternalInput").ap()
    yT = nc.dram_tensor("yT", [D, T], BF16, kind="ExternalOutput").ap()

    with tile.TileContext(nc) as tc, ExitStack() as ctx:
        _body(ctx, tc, xT, wq, wk, wv, wo, bq, bk, maskTd, identd, yT)
    nc.compile()
    _CACHE["nc"] = nc
    return nc


def _host_consts():
    p = np.arange(128)[:, None]
    f = np.arange(128)[None, :]
    maskT = np.where(f >= p, 1.0, 0.0).astype(ml_dtypes.bfloat16)
    ident = np.eye(128, dtype=ml_dtypes.bfloat16)
    return maskT, ident


def make_in_maps(x, Wq, bq, Wk, bk, Wv, bv, Wo, bo):
    maskT, ident = _host_consts()
    bf = lambda a: np.ascontiguousarray(a).astype(ml_dtypes.bfloat16)

    xTb = [bf(x[b].T) for b in range(2)]
    in_maps = []
    for c in range(8):
        b, g = divmod(c, G)
        in_maps.append({
            "xT": xTb[b],
            "wq": bf(Wq[:, g * QC:(g + 1) * QC]),
            "wk": bf(Wk[:, g * DH:(g + 1) * DH]),
            "wv": bf(Wv[:, g * DH:(g + 1) * DH]),
            "wo": bf(Wo[g * QC:(g + 1) * QC, :]),
            "bq": np.ascontiguousarray(bq[g * QC:(g + 1) * QC]),
            "bk": np.ascontiguousarray(bk[g * DH:(g + 1) * DH]),
            "maskT": maskT,
            "ident": ident,
        })
    return in_maps


def kernel(x, Wq, bq, Wk, bk, Wv, bv, Wo, bo):
    global LAST_RESULTS
    x = np.asarray(x, np.float32)
    Wq = np.asarray(Wq, np.float32)
    Wk = np.asarray(Wk, np.float32)
    Wv = np.asarray(Wv, np.float32)
    Wo = np.asarray(Wo, np.float32)
    bq = np.asarray(bq, np.float32)
    bk = np.asarray(bk, np.float32)
    bv = np.asarray(bv, np.float32)
    bo = np.asarray(bo, np.float32)

    nc = _build_nc()
    in_maps = make_in_maps(x, Wq, bq, Wk, bk, Wv, bv, Wo, bo)

    res = run_bass_kernel_spmd(nc, in_maps, list(range(8)), trace=TRACE,
                               **TRACE_KW)
    LAST_RESULTS = res

    # V bias folded: bo_eff = bo + (bv per head) @ Wo
    bv_heads = np.repeat(bv.reshape(G, DH), HPG, axis=0).reshape(-1)
    bo_eff = bo + bv_heads @ Wo

    y = np.empty((2, T, D), np.float32)
    for b in range(2):
        acc = res.results[b * G + 0]["yT"].astype(np.float32)
        for g in range(1, G):
            acc += res.results[b * G + g]["yT"].astype(np.float32)
        y[b] = acc.T + bo_eff
    return y
